# revision 8
# baseline (speedup 1.0000x reference)
"""MoE ExpertRouter kernel for 8 Trainium2 NeuronCores.

Strategy (expert-parallel): the host computes the gate (67 M-MAC, 0.05%
of total FLOPs), does top-k routing + softmax weights, and distributes
(token, expert) pairs to cores. Each core holds the FULL FFN weights of
TWO experts resident in SBUF (bf16: 2 x 8 MB), and runs two token
streams:
    stream A: CA tokens through expert eA[core]
    stream B: CB tokens through expert eB[core]
Capacities (CA, CB) are solved per input so that 8(CA+CB) covers the
16 (expert -> slot) pieces with minimal padding; every core does exactly
CA+CB token-FFNs, so the load is balanced to <1% of the 2048/core ideal
(vs ~6% padding waste for the fixed hot/cold-pair scheme).

All matmul operands are bf16 (same PE rate as fp32r, half the DMA/SBUF
traffic, no >=256 moving-dim constraint; measured RMS error 3.7e-3 vs
the 2e-2 budget). When b1 == b2 == 0 (always, per this module's reference), the
softmax weight p is folded into x on the host: relu(p*x@w1)@w2 =
p*(relu(x@w1)@w2), so the device does no per-token scaling at all and
the returned yT partials are final; the host just scatter-adds them.

Device layout avoids all transposes:
    mm1: hT[f,c] = sum_k w1[k,f] * xT[k,c]   (lhsT = w1 tile, rhs = xT)
    mm2: yT[d,c] = sum_f w2[f,d] * hT[f,c]   (lhsT = w2 tile, rhs = hT)
"""

import itertools
import sys

try:
    import concourse.bass as bass
except ImportError:  # pragma: no cover
    sys.path.insert(0, "/opt/trn_rl_repo")
    import concourse.bass as bass

import numpy as np
import bass_rust
import concourse.mybir as mybir
from concourse.tile import TileContext
from concourse.bass_utils import run_bass_kernel_spmd

P = 128
D_MODEL = 1024
D_FF = 2048
N_EXPERTS = 8
N_CORES = 8
KO = D_MODEL // P   # 8  k-tiles for mm1
FO = D_FF // P      # 16 f-tiles
DO = D_MODEL // P   # 8  d-tiles for mm2
PSUM_W = 512        # fp32 columns per PSUM bank
NEG_INF = -1e9

F32 = mybir.dt.float32
BF16 = mybir.dt.bfloat16
NP_BF16 = mybir.dt.np(BF16)

_nc_cache = {}


def _split_multiwait(nc):
    """The walrus in this env allows a single sync-wait per instruction;
    Tile's tail drain carries several. Hoist extras onto single-wait NOPs
    inserted immediately before the offending instruction."""
    k = 0
    for f in nc.m.functions:
        for b in f.blocks:
            out, changed = [], False
            for inst in b.instructions:
                si = inst.sync_info
                if si is not None and si.on_wait and len(si.on_wait) > 1:
                    waits = list(si.on_wait)
                    for w in waits[:-1]:
                        nop = bass_rust.InstNoOp(
                            name=f"I-splitw-{k}", ins=[], outs=[]
                        )
                        k += 1
                        nop.engine = inst.engine
                        nop.sync_info = mybir.SyncInfo(on_wait=[w], on_update=[])
                        out.append(nop)
                    inst.sync_info = mybir.SyncInfo(
                        on_wait=[waits[-1]], on_update=list(si.on_update)
                    )
                    changed = True
                out.append(inst)
            if changed:
                b.instructions = out


def _chunks(C):
    """Split C greedily into 512-wide chunks plus one ragged tail
    (PSUM bank = 512 fp32). Interleaved A/B on hardware measured greedy-512
    ~4% faster than near-even widths: narrow matmuls pay a per-column
    penalty (LDWEIGHTS pipelining margin) that favors concentrating width.
    """
    out, c0 = [], 0
    while C - c0 > 0:
        w = min(PSUM_W, C - c0)
        out.append((c0, w))
        c0 += w
    return out


def _build_nc(CA, CB, use_b1, use_b2, fold_p, repeat=1):
    nc = bass.Bass()
    xTa = nc.declare_dram_parameter("xTa", [D_MODEL, CA], BF16, isOutput=False)
    xTb = nc.declare_dram_parameter("xTb", [D_MODEL, CB], BF16, isOutput=False)
    w1a = nc.declare_dram_parameter("w1a", [D_MODEL, D_FF], BF16, isOutput=False)
    w2a = nc.declare_dram_parameter("w2a", [D_FF, D_MODEL], BF16, isOutput=False)
    w1b = nc.declare_dram_parameter("w1b", [D_MODEL, D_FF], BF16, isOutput=False)
    w2b = nc.declare_dram_parameter("w2b", [D_FF, D_MODEL], BF16, isOutput=False)
    if not fold_p:
        wta = nc.declare_dram_parameter("wta", [P, CA], F32, isOutput=False)
        wtb = nc.declare_dram_parameter("wtb", [P, CB], F32, isOutput=False)
    if use_b1:
        b1ca = nc.declare_dram_parameter("b1ca", [P, FO], F32, isOutput=False)
        b1cb = nc.declare_dram_parameter("b1cb", [P, FO], F32, isOutput=False)
    if use_b2:
        b2ca = nc.declare_dram_parameter("b2ca", [P, DO], F32, isOutput=False)
        b2cb = nc.declare_dram_parameter("b2cb", [P, DO], F32, isOutput=False)
    y_dt = BF16 if fold_p else F32
    yTa = nc.declare_dram_parameter("yTa", [D_MODEL, CA], y_dt, isOutput=True)
    yTb = nc.declare_dram_parameter("yTb", [D_MODEL, CB], y_dt, isOutput=True)

    relu = mybir.ActivationFunctionType.Relu

    with TileContext(nc) as tc:
        with (
            tc.tile_pool(name="wpool", bufs=1) as wpool,
            tc.tile_pool(name="xpool", bufs=2) as xpool,
            tc.tile_pool(name="hpool", bufs=2) as hpool,
            tc.tile_pool(name="ypool", bufs=3) as ypool,
            tc.tile_pool(name="ps1", bufs=4, space="PSUM") as pspool1,
            tc.tile_pool(name="ps2", bufs=4, space="PSUM") as pspool2,
        ):
            def load_stream(tag, xT, w1, w2, C, first):
                xTr = xT.ap().rearrange("(ko p) c -> p ko c", p=P)
                w1r = w1.ap().rearrange("(ko p) f -> p ko f", p=P)
                w2r = w2.ap().rearrange("(fo p) d -> p fo d", p=P)
                w1s = [
                    wpool.tile([P, D_FF], BF16, tag=f"w1{tag}{ko}", name=f"w1{tag}{ko}")
                    for ko in range(KO)
                ]
                for ko in range(KO):
                    nc.sync.dma_start(w1s[ko][:], w1r[:, ko, :])
                xt0 = None
                if first:
                    # first chunk's activations land before the 24MB of
                    # remaining weights so mm1 isn't stuck at launch
                    cw0 = _chunks(C)[0][1]
                    xt0 = xpool.tile([P, KO, PSUM_W], BF16, tag="xt", name="xt")[:, :, :cw0]
                    nc.sync.dma_start(xt0, xTr[:, :, 0:cw0])
                w2s = [
                    wpool.tile([P, D_MODEL], BF16, tag=f"w2{tag}{fo}", name=f"w2{tag}{fo}")
                    for fo in range(FO)
                ]
                for fo in range(FO):
                    nc.sync.dma_start(w2s[fo][:], w2r[:, fo, :])
                return xTr, w1s, w2s, xt0

            sa = load_stream("a", xTa, w1a, w2a, CA, True)
            sb = load_stream("b", xTb, w1b, w2b, CB, False)

            def load_aux(name, par, cols):
                t = wpool.tile([P, cols], F32, tag=name, name=name)
                nc.sync.dma_start(t[:], par.ap())
                return t

            wts_a = load_aux("wtsa", wta, CA) if not fold_p else None
            wts_b = load_aux("wtsb", wtb, CB) if not fold_p else None
            b1s_a = load_aux("b1sa", b1ca, FO) if use_b1 else None
            b1s_b = load_aux("b1sb", b1cb, FO) if use_b1 else None
            b2s_a = load_aux("b2sa", b2ca, DO) if use_b2 else None
            b2s_b = load_aux("b2sb", b2cb, DO) if use_b2 else None

            yTar = yTa.ap().rearrange("(do p) c -> p do c", p=P)
            yTbr = yTb.ap().rearrange("(do p) c -> p do c", p=P)

            def stream_body(stream, yTr, C, wts, b1s, b2s, first_xt=None):
                xTr, w1s, w2s, _ = stream
                for ci, (c0, cw) in enumerate(_chunks(C)):
                    if ci == 0 and first_xt is not None:
                        xt = first_xt
                    else:
                        xt = xpool.tile([P, KO, PSUM_W], BF16, tag="xt", name="xt")[:, :, :cw]
                        nc.sync.dma_start(xt, xTr[:, :, c0 : c0 + cw])
                    ht = hpool.tile([P, FO, PSUM_W], BF16, tag="ht", name="ht")[:, :, :cw]
                    for fo in range(FO):
                        ps = pspool1.tile([P, PSUM_W], F32, tag="ps1", name="ps1")[:, :cw]
                        for ko in range(KO):
                            nc.tensor.matmul(
                                ps,
                                w1s[ko][:, fo * P : (fo + 1) * P],
                                xt[:, ko, :],
                                start=(ko == 0),
                                stop=(ko == KO - 1),
                            )
                        if use_b1:
                            nc.scalar.activation(
                                ht[:, fo, :], ps, relu, bias=b1s[:, fo : fo + 1]
                            )
                        else:
                            nc.scalar.activation(ht[:, fo, :], ps, relu)
                    for do in range(DO):
                        ps2 = pspool2.tile([P, PSUM_W], F32, tag="ps2", name="ps2")[:, :cw]
                        for fo in range(FO):
                            nc.tensor.matmul(
                                ps2,
                                w2s[fo][:, do * P : (do + 1) * P],
                                ht[:, fo, :],
                                start=(fo == 0),
                                stop=(fo == FO - 1),
                            )
                        yt = ypool.tile([P, PSUM_W], y_dt, tag="yt", name="yt")[:, :cw]
                        if fold_p:
                            nc.vector.tensor_copy(yt, ps2)
                        else:
                            if use_b2:
                                nc.vector.tensor_scalar_add(yt, ps2, b2s[:, do : do + 1])
                                nc.vector.tensor_mul(yt, yt, wts[:, c0 : c0 + cw])
                            else:
                                nc.vector.tensor_mul(yt, ps2, wts[:, c0 : c0 + cw])
                        nc.sync.dma_start(yTr[:, do, c0 : c0 + cw], yt)

            def full_pass(first=False):
                stream_body(sa, yTar, CA, wts_a, b1s_a, b2s_a,
                            first_xt=sa[3] if first else None)
                stream_body(sb, yTbr, CB, wts_b, b1s_b, b2s_b)

            if repeat > 1:
                # hardware loop around the steady-state pass, used only for
                # benchmarking (delta-timing across repeat counts)
                full_pass(first=True)
                with tc.For_i(0, repeat - 1, 1):
                    full_pass()
            else:
                full_pass(first=True)

    _split_multiwait(nc)
    return nc


def _solve_caps(cnts):
    """Pick slot capacities (CA, CB) and per-expert piece patterns
    (a_e, b_e) with sum(a) = sum(b) = 8 and a_e*CA + b_e*CB >= cnt_e,
    minimizing per-core work CA + CB."""
    E = len(cnts)
    best = None
    if E == 8:
        for pat in itertools.product(((2, 0), (1, 1), (0, 2)), repeat=8):
            if sum(p[0] for p in pat) != 8 or sum(p[1] for p in pat) != 8:
                continue
            loA = max((-(-c // 2) for c, p in zip(cnts, pat) if p == (2, 0)), default=0)
            loB = max((-(-c // 2) for c, p in zip(cnts, pat) if p == (0, 2)), default=0)
            mid = max((c for c, p in zip(cnts, pat) if p == (1, 1)), default=0)
            tot = max(loA + loB, mid)
            if best is None or tot < best[0]:
                CB = max(loB, 16)
                CA = max(loA, tot - CB, 16)
                CB = max(CB, tot - CA)
                best = (CA + CB, CA, CB, pat)
    if best is None:
        # fallback: equal capacities C with multi-piece packing
        lo, hi = max(1, max(cnts) // 16), max(cnts)
        while lo < hi:
            mid_ = (lo + hi) // 2
            if sum(-(-c // mid_) for c in cnts) <= 2 * N_CORES:
                hi = mid_
            else:
                lo = mid_ + 1
        C = max(lo, 16)
        pieces = []
        for e, c in enumerate(cnts):
            pieces += [e] * (-(-c // C)) if c else []
        pieces += [0] * (2 * N_CORES - len(pieces))
        pat = [(0, 0)] * E
        for i, e in enumerate(pieces):
            a, b = pat[e]
            pat[e] = (a + 1, b) if i < N_CORES else (a, b + 1)
        best = (2 * C, C, C, tuple(pat))
    return best[1], best[2], best[3]


def _unwedge():
    """A trivial PJRT op clears NRT_EXEC_UNIT_UNRECOVERABLE wedges
    (observed after abrupt teardown of a previous process)."""
    try:
        import jax
        import jax.numpy as jnp

        for d in jax.devices()[:N_CORES]:
            float(jnp.sum(jax.device_put(np.ones((2, 2), np.float32), d)))
    except Exception:
        pass


def _run_retry(nc, in_maps, core_ids):
    last = None
    for attempt in range(3):
        try:
            return run_bass_kernel_spmd(nc, in_maps, core_ids)
        except Exception as e:  # noqa: BLE001
            last = e
            _unwedge()
    raise last


def _numpy_fallback(xf, gate_w, gate_b, w1, b1, w2, b2, k, B, S, D):
    E = gate_w.shape[-1]
    logits = xf @ gate_w + gate_b
    order = np.argsort(-logits, axis=-1, kind="stable")
    sel = np.zeros(logits.shape, bool)
    np.put_along_axis(sel, order[:, :k], True, axis=-1)
    sp = np.where(sel, logits, NEG_INF)
    ew = np.exp(sp - sp.max(-1, keepdims=True))
    ew /= ew.sum(-1, keepdims=True)
    out = np.zeros_like(xf)
    for e in range(E):
        idx = np.nonzero(sel[:, e])[0]
        h = np.maximum(xf[idx] @ w1[e] + b1[e], 0.0)
        out[idx] += ew[idx, e][:, None] * (h @ w2[e] + b2[e])
    return out.reshape(B, S, D)


def kernel(x, gate_w, gate_b, w1, b1, w2, b2, top_k):
    x = np.asarray(x, np.float32)
    gate_w = np.asarray(gate_w, np.float32)
    gate_b = np.asarray(gate_b, np.float32)
    w1 = np.ascontiguousarray(np.asarray(w1, np.float32))
    b1 = np.asarray(b1, np.float32)
    w2 = np.ascontiguousarray(np.asarray(w2, np.float32))
    b2 = np.asarray(b2, np.float32)
    k = int(top_k)

    B, S, D = x.shape
    E = gate_w.shape[-1]
    T = B * S
    xf = np.ascontiguousarray(x.reshape(T, D))

    if (
        D != D_MODEL
        or E != N_EXPERTS
        or w1.shape != (E, D_MODEL, D_FF)
        or w2.shape != (E, D_FF, D_MODEL)
        or k < 1
    ):
        return _numpy_fallback(xf, gate_w, gate_b, w1, b1, w2, b2, k, B, S, D)

    # --- host routing (the shard step) ---
    # fp64 gate for tie-stable top-k: verified to match fp32 jax top_k
    logits64 = xf.astype(np.float64) @ gate_w.astype(np.float64) + gate_b
    order = np.argsort(-logits64, axis=-1, kind="stable")
    topk = order[:, :k]  # [T, k]
    selected = np.zeros((T, E), bool)
    np.put_along_axis(selected, topk, True, axis=-1)
    sparse = np.where(selected, logits64, NEG_INF)
    m = sparse.max(axis=-1, keepdims=True)
    ew = np.exp(sparse - m)
    ew /= ew.sum(axis=-1, keepdims=True)  # [T, E]; exactly 0 off the top-k
    ewf = ew.astype(np.float32)

    idx = [np.nonzero(selected[:, e])[0] for e in range(E)]
    cnts = [len(i) for i in idx]

    use_b1 = bool(np.any(b1))
    use_b2 = bool(np.any(b2))
    fold_p = not use_b1 and not use_b2

    CA, CB, pat = _solve_caps(cnts)

    # cut each expert's token list into its assigned slot pieces
    pieces_a, pieces_b = [], []
    for e, (a_e, b_e) in enumerate(pat):
        pos, n = 0, cnts[e]
        for _ in range(a_e):
            take = max(0, min(CA, n - pos))
            pieces_a.append((e, pos, take))
            pos += take
        for _ in range(b_e):
            take = max(0, min(CB, n - pos))
            pieces_b.append((e, pos, take))
            pos += take
        assert pos >= n, (e, pat, CA, CB, cnts)
    assert len(pieces_a) == N_CORES and len(pieces_b) == N_CORES

    global last_cfg
    last_cfg = dict(CA=CA, CB=CB, use_b1=use_b1, use_b2=use_b2, fold_p=fold_p)

    key = (CA, CB, use_b1, use_b2, fold_p)
    if key not in _nc_cache:
        _nc_cache[key] = _build_nc(CA, CB, use_b1, use_b2, fold_p)
    nc = _nc_cache[key]

    w1h = w1.astype(NP_BF16)
    w2h = w2.astype(NP_BF16)

    def pack_x(e, start, ln, C):
        out = np.zeros((D_MODEL, C), NP_BF16)
        if ln:
            ia = idx[e][start : start + ln]
            cols = xf[ia]
            if fold_p:
                cols = cols * ewf[ia, e][:, None]
            out[:, :ln] = cols.T.astype(NP_BF16)
        return out

    def pack_wt(e, start, ln, C):
        out = np.zeros((P, C), np.float32)
        if ln:
            ia = idx[e][start : start + ln]
            out[:, :ln] = ewf[ia, e][None, :]
        return out

    colmaj = lambda v, n: np.ascontiguousarray(v.reshape(n, P).T)

    in_maps = []
    for c in range(N_CORES):
        ea, sa_, la = pieces_a[c]
        eb, sb_, lb = pieces_b[c]
        m = {
            "xTa": pack_x(ea, sa_, la, CA),
            "xTb": pack_x(eb, sb_, lb, CB),
            "w1a": w1h[ea],
            "w2a": w2h[ea],
            "w1b": w1h[eb],
            "w2b": w2h[eb],
        }
        if not fold_p:
            m["wta"] = pack_wt(ea, sa_, la, CA)
            m["wtb"] = pack_wt(eb, sb_, lb, CB)
        if use_b1:
            m["b1ca"] = colmaj(b1[ea], FO)
            m["b1cb"] = colmaj(b1[eb], FO)
        if use_b2:
            m["b2ca"] = colmaj(b2[ea], DO)
            m["b2cb"] = colmaj(b2[eb], DO)
        in_maps.append(m)

    res = _run_retry(nc, in_maps, list(range(N_CORES)))

    # --- unshard: scatter-add weighted expert partials ---
    out = np.zeros((T, D), np.float32)
    for c in range(N_CORES):
        ea, sa_, la = pieces_a[c]
        eb, sb_, lb = pieces_b[c]
        if la:
            out[idx[ea][sa_ : sa_ + la]] += (
                res.results[c]["yTa"][:, :la].astype(np.float32).T
            )
        if lb:
            out[idx[eb][sb_ : sb_ + lb]] += (
                res.results[c]["yTb"][:, :lb].astype(np.float32).T
            )
    return out.reshape(B, S, D)


# config of the last kernel() call, for test harness repeat-builds
last_cfg = None


# revision 9
# speedup vs baseline: 1.0220x; 1.0220x over previous
"""MoE ExpertRouter kernel for 8 Trainium2 NeuronCores.

Strategy (expert-parallel): the host computes the gate (67 M-MAC, 0.05%
of total FLOPs), does top-k routing + softmax weights, and distributes
(token, expert) pairs to cores. Each core holds the FULL FFN weights of
TWO experts resident in SBUF (bf16: 2 x 8 MB), and runs two token
streams:
    stream A: CA tokens through expert eA[core]
    stream B: CB tokens through expert eB[core]
Capacities (CA, CB) are solved per input so that 8(CA+CB) covers the
16 (expert -> slot) pieces with minimal padding; every core does exactly
CA+CB token-FFNs, so the load is balanced to <1% of the 2048/core ideal
(vs ~6% padding waste for the fixed hot/cold-pair scheme).

All matmul operands are bf16 (same PE rate as fp32r, half the DMA/SBUF
traffic, no >=256 moving-dim constraint; measured RMS error 3.7e-3 vs
the 2e-2 budget). When b1 == b2 == 0 (always, per this module's reference), the
softmax weight p is folded into x on the host: relu(p*x@w1)@w2 =
p*(relu(x@w1)@w2), so the device does no per-token scaling at all and
the returned yT partials are final; the host just scatter-adds them.

Device layout avoids all transposes:
    mm1: hT[f,c] = sum_k w1[k,f] * xT[k,c]   (lhsT = w1 tile, rhs = xT)
    mm2: yT[d,c] = sum_f w2[f,d] * hT[f,c]   (lhsT = w2 tile, rhs = hT)
"""

import itertools
import sys

try:
    import concourse.bass as bass
except ImportError:  # pragma: no cover
    sys.path.insert(0, "/opt/trn_rl_repo")
    import concourse.bass as bass

import numpy as np
import bass_rust
import concourse.mybir as mybir
from concourse.tile import TileContext
from concourse.bass_utils import run_bass_kernel_spmd

P = 128
D_MODEL = 1024
D_FF = 2048
N_EXPERTS = 8
N_CORES = 8
KO = D_MODEL // P   # 8  k-tiles for mm1
FO = D_FF // P      # 16 f-tiles
DO = D_MODEL // P   # 8  d-tiles for mm2
PSUM_W = 512        # fp32 columns per PSUM bank
NEG_INF = -1e9

F32 = mybir.dt.float32
BF16 = mybir.dt.bfloat16
NP_BF16 = mybir.dt.np(BF16)

_nc_cache = {}


def _split_multiwait(nc):
    """The walrus in this env allows a single sync-wait per instruction;
    Tile's tail drain carries several. Hoist extras onto single-wait NOPs
    inserted immediately before the offending instruction."""
    k = 0
    for f in nc.m.functions:
        for b in f.blocks:
            out, changed = [], False
            for inst in b.instructions:
                si = inst.sync_info
                if si is not None and si.on_wait and len(si.on_wait) > 1:
                    waits = list(si.on_wait)
                    for w in waits[:-1]:
                        nop = bass_rust.InstNoOp(
                            name=f"I-splitw-{k}", ins=[], outs=[]
                        )
                        k += 1
                        nop.engine = inst.engine
                        nop.sync_info = mybir.SyncInfo(on_wait=[w], on_update=[])
                        out.append(nop)
                    inst.sync_info = mybir.SyncInfo(
                        on_wait=[waits[-1]], on_update=list(si.on_update)
                    )
                    changed = True
                out.append(inst)
            if changed:
                b.instructions = out


def _chunks(C):
    """Split C into 512-wide chunks plus a tail (PSUM bank = 512 fp32).
    Interleaved same-power-state A/B on hardware: mid-width chunks
    (~300-480 cols) pay a hidden per-pass penalty the cost model misses
    ([512,512,56] beat [360,360,360] by ~25us/pass; [494,493] beat
    [512,475] by ~14us/pass). So: greedy 512s, but when the ragged tail
    lands in the bad zone, rebalance the last two chunks to ~half of
    (512+tail), keeping both >= ~484."""
    widths, left = [], C
    while left > 0:
        w = min(PSUM_W, left)
        widths.append(w)
        left -= w
    if len(widths) >= 2 and 456 <= widths[-1] < PSUM_W:
        s = widths[-2] + widths[-1]
        widths[-2], widths[-1] = (s + 1) // 2, s // 2
    out, c0 = [], 0
    for w in widths:
        out.append((c0, w))
        c0 += w
    return out


def _build_nc(CA, CB, use_b1, use_b2, fold_p, repeat=1):
    nc = bass.Bass()
    xTa = nc.declare_dram_parameter("xTa", [D_MODEL, CA], BF16, isOutput=False)
    xTb = nc.declare_dram_parameter("xTb", [D_MODEL, CB], BF16, isOutput=False)
    w1a = nc.declare_dram_parameter("w1a", [D_MODEL, D_FF], BF16, isOutput=False)
    w2a = nc.declare_dram_parameter("w2a", [D_FF, D_MODEL], BF16, isOutput=False)
    w1b = nc.declare_dram_parameter("w1b", [D_MODEL, D_FF], BF16, isOutput=False)
    w2b = nc.declare_dram_parameter("w2b", [D_FF, D_MODEL], BF16, isOutput=False)
    if not fold_p:
        wta = nc.declare_dram_parameter("wta", [P, CA], F32, isOutput=False)
        wtb = nc.declare_dram_parameter("wtb", [P, CB], F32, isOutput=False)
    if use_b1:
        b1ca = nc.declare_dram_parameter("b1ca", [P, FO], F32, isOutput=False)
        b1cb = nc.declare_dram_parameter("b1cb", [P, FO], F32, isOutput=False)
    if use_b2:
        b2ca = nc.declare_dram_parameter("b2ca", [P, DO], F32, isOutput=False)
        b2cb = nc.declare_dram_parameter("b2cb", [P, DO], F32, isOutput=False)
    y_dt = BF16 if fold_p else F32
    yTa = nc.declare_dram_parameter("yTa", [D_MODEL, CA], y_dt, isOutput=True)
    yTb = nc.declare_dram_parameter("yTb", [D_MODEL, CB], y_dt, isOutput=True)

    relu = mybir.ActivationFunctionType.Relu

    with TileContext(nc) as tc:
        with (
            tc.tile_pool(name="wpool", bufs=1) as wpool,
            tc.tile_pool(name="xpool", bufs=2) as xpool,
            tc.tile_pool(name="hpool", bufs=2) as hpool,
            tc.tile_pool(name="ypool", bufs=3) as ypool,
            tc.tile_pool(name="ps1", bufs=4, space="PSUM") as pspool1,
            tc.tile_pool(name="ps2", bufs=4, space="PSUM") as pspool2,
        ):
            def load_stream(tag, xT, w1, w2, C, first):
                xTr = xT.ap().rearrange("(ko p) c -> p ko c", p=P)
                w1r = w1.ap().rearrange("(ko p) f -> p ko f", p=P)
                w2r = w2.ap().rearrange("(fo p) d -> p fo d", p=P)
                w1s = [
                    wpool.tile([P, D_FF], BF16, tag=f"w1{tag}{ko}", name=f"w1{tag}{ko}")
                    for ko in range(KO)
                ]
                for ko in range(KO):
                    nc.sync.dma_start(w1s[ko][:], w1r[:, ko, :])
                xt0 = None
                if first:
                    # first chunk's activations land before the 24MB of
                    # remaining weights so mm1 isn't stuck at launch
                    cw0 = _chunks(C)[0][1]
                    xt0 = xpool.tile([P, KO, PSUM_W], BF16, tag="xt", name="xt")[:, :, :cw0]
                    nc.sync.dma_start(xt0, xTr[:, :, 0:cw0])
                w2s = [
                    wpool.tile([P, D_MODEL], BF16, tag=f"w2{tag}{fo}", name=f"w2{tag}{fo}")
                    for fo in range(FO)
                ]
                for fo in range(FO):
                    nc.sync.dma_start(w2s[fo][:], w2r[:, fo, :])
                return xTr, w1s, w2s, xt0

            sa = load_stream("a", xTa, w1a, w2a, CA, True)
            sb = load_stream("b", xTb, w1b, w2b, CB, False)

            def load_aux(name, par, cols):
                t = wpool.tile([P, cols], F32, tag=name, name=name)
                nc.sync.dma_start(t[:], par.ap())
                return t

            wts_a = load_aux("wtsa", wta, CA) if not fold_p else None
            wts_b = load_aux("wtsb", wtb, CB) if not fold_p else None
            b1s_a = load_aux("b1sa", b1ca, FO) if use_b1 else None
            b1s_b = load_aux("b1sb", b1cb, FO) if use_b1 else None
            b2s_a = load_aux("b2sa", b2ca, DO) if use_b2 else None
            b2s_b = load_aux("b2sb", b2cb, DO) if use_b2 else None

            yTar = yTa.ap().rearrange("(do p) c -> p do c", p=P)
            yTbr = yTb.ap().rearrange("(do p) c -> p do c", p=P)

            def stream_body(stream, yTr, C, wts, b1s, b2s, first_xt=None):
                xTr, w1s, w2s, _ = stream
                for ci, (c0, cw) in enumerate(_chunks(C)):
                    if ci == 0 and first_xt is not None:
                        xt = first_xt
                    else:
                        xt = xpool.tile([P, KO, PSUM_W], BF16, tag="xt", name="xt")[:, :, :cw]
                        nc.sync.dma_start(xt, xTr[:, :, c0 : c0 + cw])
                    ht = hpool.tile([P, FO, PSUM_W], BF16, tag="ht", name="ht")[:, :, :cw]
                    for fo in range(FO):
                        ps = pspool1.tile([P, PSUM_W], F32, tag="ps1", name="ps1")[:, :cw]
                        for ko in range(KO):
                            nc.tensor.matmul(
                                ps,
                                w1s[ko][:, fo * P : (fo + 1) * P],
                                xt[:, ko, :],
                                start=(ko == 0),
                                stop=(ko == KO - 1),
                            )
                        if use_b1:
                            nc.scalar.activation(
                                ht[:, fo, :], ps, relu, bias=b1s[:, fo : fo + 1]
                            )
                        else:
                            nc.scalar.activation(ht[:, fo, :], ps, relu)
                    for do in range(DO):
                        ps2 = pspool2.tile([P, PSUM_W], F32, tag="ps2", name="ps2")[:, :cw]
                        for fo in range(FO):
                            nc.tensor.matmul(
                                ps2,
                                w2s[fo][:, do * P : (do + 1) * P],
                                ht[:, fo, :],
                                start=(fo == 0),
                                stop=(fo == FO - 1),
                            )
                        yt = ypool.tile([P, PSUM_W], y_dt, tag="yt", name="yt")[:, :cw]
                        if fold_p:
                            nc.vector.tensor_copy(yt, ps2)
                        else:
                            if use_b2:
                                nc.vector.tensor_scalar_add(yt, ps2, b2s[:, do : do + 1])
                                nc.vector.tensor_mul(yt, yt, wts[:, c0 : c0 + cw])
                            else:
                                nc.vector.tensor_mul(yt, ps2, wts[:, c0 : c0 + cw])
                        nc.sync.dma_start(yTr[:, do, c0 : c0 + cw], yt)

            def full_pass(first=False):
                stream_body(sa, yTar, CA, wts_a, b1s_a, b2s_a,
                            first_xt=sa[3] if first else None)
                stream_body(sb, yTbr, CB, wts_b, b1s_b, b2s_b)

            if repeat > 1:
                # hardware loop around the steady-state pass, used only for
                # benchmarking (delta-timing across repeat counts)
                full_pass(first=True)
                with tc.For_i(0, repeat - 1, 1):
                    full_pass()
            else:
                full_pass(first=True)

    _split_multiwait(nc)
    return nc


def _solve_caps(cnts):
    """Pick slot capacities (CA, CB) and per-expert piece patterns
    (a_e, b_e) with sum(a) = sum(b) = 8 and a_e*CA + b_e*CB >= cnt_e,
    minimizing per-core work CA + CB."""
    E = len(cnts)
    best = None
    if E == 8:
        for pat in itertools.product(((2, 0), (1, 1), (0, 2)), repeat=8):
            if sum(p[0] for p in pat) != 8 or sum(p[1] for p in pat) != 8:
                continue
            loA = max((-(-c // 2) for c, p in zip(cnts, pat) if p == (2, 0)), default=0)
            loB = max((-(-c // 2) for c, p in zip(cnts, pat) if p == (0, 2)), default=0)
            mid = max((c for c, p in zip(cnts, pat) if p == (1, 1)), default=0)
            tot = max(loA + loB, mid)
            if best is None or tot < best[0]:
                CB = max(loB, 16)
                CA = max(loA, tot - CB, 16)
                CB = max(CB, tot - CA)
                best = (CA + CB, CA, CB, pat)
    if best is None:
        # fallback: equal capacities C with multi-piece packing
        lo, hi = max(1, max(cnts) // 16), max(cnts)
        while lo < hi:
            mid_ = (lo + hi) // 2
            if sum(-(-c // mid_) for c in cnts) <= 2 * N_CORES:
                hi = mid_
            else:
                lo = mid_ + 1
        C = max(lo, 16)
        pieces = []
        for e, c in enumerate(cnts):
            pieces += [e] * (-(-c // C)) if c else []
        pieces += [0] * (2 * N_CORES - len(pieces))
        pat = [(0, 0)] * E
        for i, e in enumerate(pieces):
            a, b = pat[e]
            pat[e] = (a + 1, b) if i < N_CORES else (a, b + 1)
        best = (2 * C, C, C, tuple(pat))
    return best[1], best[2], best[3]


def _unwedge():
    """A trivial PJRT op clears NRT_EXEC_UNIT_UNRECOVERABLE wedges
    (observed after abrupt teardown of a previous process)."""
    try:
        import jax
        import jax.numpy as jnp

        for d in jax.devices()[:N_CORES]:
            float(jnp.sum(jax.device_put(np.ones((2, 2), np.float32), d)))
    except Exception:
        pass


def _run_retry(nc, in_maps, core_ids):
    last = None
    for attempt in range(3):
        try:
            return run_bass_kernel_spmd(nc, in_maps, core_ids)
        except Exception as e:  # noqa: BLE001
            last = e
            _unwedge()
    raise last


def _numpy_fallback(xf, gate_w, gate_b, w1, b1, w2, b2, k, B, S, D):
    E = gate_w.shape[-1]
    logits = xf @ gate_w + gate_b
    order = np.argsort(-logits, axis=-1, kind="stable")
    sel = np.zeros(logits.shape, bool)
    np.put_along_axis(sel, order[:, :k], True, axis=-1)
    sp = np.where(sel, logits, NEG_INF)
    ew = np.exp(sp - sp.max(-1, keepdims=True))
    ew /= ew.sum(-1, keepdims=True)
    out = np.zeros_like(xf)
    for e in range(E):
        idx = np.nonzero(sel[:, e])[0]
        h = np.maximum(xf[idx] @ w1[e] + b1[e], 0.0)
        out[idx] += ew[idx, e][:, None] * (h @ w2[e] + b2[e])
    return out.reshape(B, S, D)


def kernel(x, gate_w, gate_b, w1, b1, w2, b2, top_k):
    x = np.asarray(x, np.float32)
    gate_w = np.asarray(gate_w, np.float32)
    gate_b = np.asarray(gate_b, np.float32)
    w1 = np.ascontiguousarray(np.asarray(w1, np.float32))
    b1 = np.asarray(b1, np.float32)
    w2 = np.ascontiguousarray(np.asarray(w2, np.float32))
    b2 = np.asarray(b2, np.float32)
    k = int(top_k)

    B, S, D = x.shape
    E = gate_w.shape[-1]
    T = B * S
    xf = np.ascontiguousarray(x.reshape(T, D))

    if (
        D != D_MODEL
        or E != N_EXPERTS
        or w1.shape != (E, D_MODEL, D_FF)
        or w2.shape != (E, D_FF, D_MODEL)
        or k < 1
    ):
        return _numpy_fallback(xf, gate_w, gate_b, w1, b1, w2, b2, k, B, S, D)

    # --- host routing (the shard step) ---
    # fp64 gate for tie-stable top-k: verified to match fp32 jax top_k
    logits64 = xf.astype(np.float64) @ gate_w.astype(np.float64) + gate_b
    order = np.argsort(-logits64, axis=-1, kind="stable")
    topk = order[:, :k]  # [T, k]
    selected = np.zeros((T, E), bool)
    np.put_along_axis(selected, topk, True, axis=-1)
    sparse = np.where(selected, logits64, NEG_INF)
    m = sparse.max(axis=-1, keepdims=True)
    ew = np.exp(sparse - m)
    ew /= ew.sum(axis=-1, keepdims=True)  # [T, E]; exactly 0 off the top-k
    ewf = ew.astype(np.float32)

    idx = [np.nonzero(selected[:, e])[0] for e in range(E)]
    cnts = [len(i) for i in idx]

    use_b1 = bool(np.any(b1))
    use_b2 = bool(np.any(b2))
    fold_p = not use_b1 and not use_b2

    CA, CB, pat = _solve_caps(cnts)

    # cut each expert's token list into its assigned slot pieces
    pieces_a, pieces_b = [], []
    for e, (a_e, b_e) in enumerate(pat):
        pos, n = 0, cnts[e]
        for _ in range(a_e):
            take = max(0, min(CA, n - pos))
            pieces_a.append((e, pos, take))
            pos += take
        for _ in range(b_e):
            take = max(0, min(CB, n - pos))
            pieces_b.append((e, pos, take))
            pos += take
        assert pos >= n, (e, pat, CA, CB, cnts)
    assert len(pieces_a) == N_CORES and len(pieces_b) == N_CORES

    global last_cfg
    last_cfg = dict(CA=CA, CB=CB, use_b1=use_b1, use_b2=use_b2, fold_p=fold_p)

    key = (CA, CB, use_b1, use_b2, fold_p)
    if key not in _nc_cache:
        _nc_cache[key] = _build_nc(CA, CB, use_b1, use_b2, fold_p)
    nc = _nc_cache[key]

    w1h = w1.astype(NP_BF16)
    w2h = w2.astype(NP_BF16)

    def pack_x(e, start, ln, C):
        out = np.zeros((D_MODEL, C), NP_BF16)
        if ln:
            ia = idx[e][start : start + ln]
            cols = xf[ia]
            if fold_p:
                cols = cols * ewf[ia, e][:, None]
            out[:, :ln] = cols.T.astype(NP_BF16)
        return out

    def pack_wt(e, start, ln, C):
        out = np.zeros((P, C), np.float32)
        if ln:
            ia = idx[e][start : start + ln]
            out[:, :ln] = ewf[ia, e][None, :]
        return out

    colmaj = lambda v, n: np.ascontiguousarray(v.reshape(n, P).T)

    in_maps = []
    for c in range(N_CORES):
        ea, sa_, la = pieces_a[c]
        eb, sb_, lb = pieces_b[c]
        m = {
            "xTa": pack_x(ea, sa_, la, CA),
            "xTb": pack_x(eb, sb_, lb, CB),
            "w1a": w1h[ea],
            "w2a": w2h[ea],
            "w1b": w1h[eb],
            "w2b": w2h[eb],
        }
        if not fold_p:
            m["wta"] = pack_wt(ea, sa_, la, CA)
            m["wtb"] = pack_wt(eb, sb_, lb, CB)
        if use_b1:
            m["b1ca"] = colmaj(b1[ea], FO)
            m["b1cb"] = colmaj(b1[eb], FO)
        if use_b2:
            m["b2ca"] = colmaj(b2[ea], DO)
            m["b2cb"] = colmaj(b2[eb], DO)
        in_maps.append(m)

    res = _run_retry(nc, in_maps, list(range(N_CORES)))

    # --- unshard: scatter-add weighted expert partials ---
    out = np.zeros((T, D), np.float32)
    for c in range(N_CORES):
        ea, sa_, la = pieces_a[c]
        eb, sb_, lb = pieces_b[c]
        if la:
            out[idx[ea][sa_ : sa_ + la]] += (
                res.results[c]["yTa"][:, :la].astype(np.float32).T
            )
        if lb:
            out[idx[eb][sb_ : sb_ + lb]] += (
                res.results[c]["yTb"][:, :lb].astype(np.float32).T
            )
    return out.reshape(B, S, D)


# config of the last kernel() call, for test harness repeat-builds
last_cfg = None
